# revision 21
# baseline (speedup 1.0000x reference)
"""Bidirectional Mamba (2 layers, B=4, L=2048, D=256) on 8 TRN2 NeuronCores.

Sharding: 8 cores = 4 batches x 2 directions; each core runs one
(batch, direction) stream end-to-end in channel-major (C, T) layout.
 - PE: all matmuls (bf16 lhsT); depthwise causal conv folded into 3
   shifted in_proj taps; dense layer conv as 3 shifted matmuls.
 - Scan: 16 per-n planes, ACT exp(-n*delta) + DVE tensor_tensor_scan
   (state = a*state + b along time), y accumulated in fp32.
 - One pair AllReduce at the layer-1/2 boundary; fwd/bwd time alignment
   via data-driven parity masks (no control flow).
Final combine (res2_f + flip(res2_b) + l1out) on host.
"""

import numpy as np

import concourse.bass as bass
import concourse.bacc as bacc
import concourse.mybir as mybir
from concourse import tile
from concourse.tile_rust import add_dep_helper
from concourse.bass_utils import run_bass_kernel_spmd

F32 = mybir.dt.float32
BF16 = mybir.dt.bfloat16
AF = mybir.ActivationFunctionType
OP = mybir.AluOpType

D_MODEL, D_STATE, D_INNER, DT_RANK = 256, 16, 512, 16
T = 2048
N_LAYERS = 2
P = 128
CPT = D_MODEL // P   # 2
DPT = D_INNER // P   # 4
FT = 512
NF = T // FT         # 4
TQ = 512             # scan quarter
NQ = T // TQ         # 4
EPS = 1e-5


def build_program(n_layers=N_LAYERS, do_cc=True, fake_cc=False):
    nc = bacc.Bacc("TRN2", target_bir_lowering=False, debug=False, num_devices=8)

    def din(name, shape, dt=F32):
        return nc.dram_tensor(name, shape, dt, kind="ExternalInput").ap()

    x_in = din("x_stream", (D_MODEL, T))
    ones_in = din("ones_lhst", (P, P))
    sa_in = din("par_a", (P, 1))
    sb_in = din("par_b", (P, 1))
    eps_in = din("epsb", (P, 1))
    W = {}
    for l in range(n_layers):
        W[l] = dict(
            wxi=din(f"wxi{l}", (3, D_MODEL, D_INNER), BF16),
            wz=din(f"wz{l}", (D_MODEL, D_INNER), BF16),
            cb=din(f"cb{l}", (D_INNER, 1)),
            wxp=din(f"wxp{l}", (D_INNER, 96), BF16),
            wdt=din(f"wdt{l}", (DT_RANK, D_INNER)),
            dtb=din(f"dtb{l}", (D_INNER, 1)),
            dvec=din(f"dvec{l}", (D_INNER, 1)),
            wout=din(f"wout{l}", (D_INNER, D_MODEL), BF16),
            wmlp=din(f"wmlp{l}", (D_MODEL, D_MODEL), BF16),
            mlpb=din(f"mlpb{l}", (D_MODEL, 1)),
            wcv=din(f"wcv{l}", (3, D_MODEL, D_MODEL), BF16),
            cvb=din(f"cvb{l}", (D_MODEL, 1)),
        )
    res2_out = nc.dram_tensor("res2", (D_MODEL, T), F32, kind="ExternalOutput").ap()
    l1_out = nc.dram_tensor("l1out", (D_MODEL, T), F32, kind="ExternalOutput").ap()
    groups = [[0, 1], [2, 3], [4, 5], [6, 7]]

    with tile.TileContext(nc) as tc:
        with (
            tc.tile_pool(name="con", bufs=1) as con,
            tc.tile_pool(name="wp", bufs=1) as wp,
            tc.tile_pool(name="act", bufs=1) as act,
            tc.tile_pool(name="tmp", bufs=1) as tmp,
            tc.tile_pool(name="sc1", bufs=1) as sc1,
            tc.tile_pool(name="sc2", bufs=2) as sc2,
            tc.tile_pool(name="ps", bufs=6, space="PSUM") as ps,
            tc.tile_pool(name="dram", bufs=1, space="DRAM") as dram,
        ):
            def dma(dst, src):
                nc.sync.dma_start(dst, src)

            def load(pool, shape, dt, src, tag):
                t_ = pool.tile(shape, dt, tag=tag)
                dma(t_[:], src)
                return t_

            ones = load(con, [P, P], F32, ones_in, "ones")
            sa = load(con, [P, 1], F32, sa_in, "sa")
            sb = load(con, [P, 1], F32, sb_in, "sb")
            epsb = load(con, [P, 1], F32, eps_in, "epsb")
            hlast = con.tile([P, D_STATE * DPT], F32, tag="hlast", name="hlast")
            obs = ps.tile([32, 32], F32, tag="obs", name="obs", bufs=1)
            pending_obs = []

            def observe(*slices):
                # tiny PE matmuls that pre-observe producer semaphores so real
                # matmuls never carry >1 sync wait (walrus S3_LW limit).
                # Ordered before the next real matmul via nosync deps.
                prev = pending_obs[-1] if pending_obs else None
                for sl in slices:
                    bi = nc.tensor.matmul(obs[0:sl.shape[1], :], sl, sl[:, 0:32],
                                          start=True, stop=True)
                    if prev is not None:
                        add_dep_helper(bi.ins, prev.ins, sync=False,
                                       reason="observer chain order")
                    pending_obs.append(bi)
                    prev = bi

            def touch(ptile):
                bi = nc.tensor.matmul(ptile[0:32, 0:32], ones[0:32, 0:32],
                                      ones[0:32, 0:32], start=True, stop=True,
                                      skip_group_check=True)
                if pending_obs:
                    add_dep_helper(bi.ins, pending_obs[-1].ins, sync=False,
                                   reason="touch after observers")
                pending_obs.append(bi)

            def MM(*args, **kw):
                bi = nc.tensor.matmul(*args, **kw)
                if pending_obs:
                    add_dep_helper(bi.ins, pending_obs[-1].ins, sync=False,
                                   reason="matmul after observers")
                    pending_obs.clear()
                return bi

            cc_in = dram.tile([P, CPT * 2 * T], F32, tag="cc_in", name="cc_in")
            cc_out = dram.tile([P, CPT * 2 * T], F32, tag="cc_out", name="cc_out")

            xres = [act.tile([P, T], F32, tag=f"xres{c}", name=f"xres{c}") for c in range(CPT)]
            for c in range(CPT):
                dma(xres[c][:], x_in[c * P:(c + 1) * P, :])

            def rmsnorm_to(dst, src):
                sq = [tmp.tile([P, T], F32, tag=f"big{c}", name=f"big{c}") for c in range(CPT)]
                for c in range(CPT):
                    nc.scalar.activation(sq[c][:], src[c][:], AF.Square)
                rn = tmp.tile([P, T], F32, tag="big2", name="big2")
                observe(sq[CPT - 1][0:32, 0:32])
                for f in range(NF):
                    s_ = slice(f * FT, (f + 1) * FT)
                    pt_ = ps.tile([P, FT], F32, tag="mm", name="mm")
                    if f > 0:
                        observe(rn[0:32, f * FT - 32:f * FT])
                    touch(pt_)
                    for c in range(CPT):
                        MM(pt_[:], ones[:], sq[c][:, s_],
                                         start=(c == 0), stop=(c == CPT - 1))
                    nc.scalar.activation(rn[:, s_], pt_[:], AF.Ln,
                                         scale=1.0 / D_MODEL, bias=epsb[:, 0:1])
                nc.scalar.activation(rn[:], rn[:], AF.Exp, scale=-0.5)
                for c in range(CPT):
                    nc.vector.tensor_tensor(dst[c][:], src[c][:], rn[:], OP.mult)
                observe(dst[CPT - 1][0:32, 0:32], rn[0:32, 0:32])

            for l in range(n_layers):
                w = W[l]
                wxi = [[load(wp, [P, D_INNER], BF16, w['wxi'][k, c * P:(c + 1) * P, :],
                             f"wxi{k}{c}") for c in range(CPT)] for k in range(3)]
                wz = [load(wp, [P, D_INNER], BF16, w['wz'][c * P:(c + 1) * P, :],
                           f"wz{c}") for c in range(CPT)]
                cb = load(wp, [P, DPT], F32,
                          w['cb'].rearrange("(d p) o -> p (d o)", p=P), "cb")
                wxp = [load(wp, [P, 96], BF16, w['wxp'][d * P:(d + 1) * P, :],
                            f"wxp{d}") for d in range(DPT)]
                wdt = load(wp, [DT_RANK, D_INNER], F32, w['wdt'], "wdt")
                dtb = load(wp, [P, DPT], F32,
                           w['dtb'].rearrange("(d p) o -> p (d o)", p=P), "dtb")
                dvec = load(wp, [P, DPT], F32,
                            w['dvec'].rearrange("(d p) o -> p (d o)", p=P), "dvec")
                wout = [load(wp, [P, D_MODEL], BF16, w['wout'][d * P:(d + 1) * P, :],
                             f"wout{d}") for d in range(DPT)]
                wmlp = [load(wp, [P, D_MODEL], BF16, w['wmlp'][c * P:(c + 1) * P, :],
                             f"wmlp{c}") for c in range(CPT)]
                mlpb = load(wp, [P, CPT], F32,
                            w['mlpb'].rearrange("(d p) o -> p (d o)", p=P), "mlpb")
                wcv = [[load(wp, [P, D_MODEL], BF16, w['wcv'][k, c * P:(c + 1) * P, :],
                             f"wcv{k}{c}") for c in range(CPT)] for k in range(3)]
                cvb = load(wp, [P, CPT], F32,
                           w['cvb'].rearrange("(d p) o -> p (d o)", p=P), "cvb")

                hidden = [act.tile([P, T], BF16, tag=f"hid{c}", name=f"hid{c}") for c in range(CPT)]
                rmsnorm_to(hidden, xres)

                # in_proj (+ folded depthwise causal conv) -> silu -> xi ; z -> silu -> zs
                xi = [act.tile([P, T], BF16, tag=f"xi{d}", name=f"xi{d}") for d in range(DPT)]
                zs = [act.tile([P, T], BF16, tag=f"zs{d}", name=f"zs{d}") for d in range(DPT)]
                observe(wxi[0][0][0:32, 0:32])
                prev_g = None
                for d in range(DPT):
                    ds_ = slice(d * P, (d + 1) * P)
                    for f in range(NF):
                        t0 = f * FT
                        pxi = ps.tile([P, FT], F32, tag="mm", name="mm")
                        if prev_g is not None:
                            pd_, pf_ = prev_g
                            observe(zs[pd_][0:32, pf_ * FT + FT - 32:pf_ * FT + FT])
                        touch(pxi)
                        prev_g = (d, f)
                        first = True
                        for k in range(3):
                            sig = 2 - k
                            for c in range(CPT):
                                lh = wxi[k][c][:, ds_]
                                if t0 - sig >= 0:
                                    MM(pxi[:], lh,
                                                     hidden[c][:, t0 - sig:t0 + FT - sig],
                                                     start=first, stop=False)
                                else:
                                    MM(pxi[:, sig:], lh,
                                                     hidden[c][:, 0:FT - sig],
                                                     start=first, stop=False)
                                first = False
                        pz = ps.tile([P, FT], F32, tag="mm", name="mm")
                        touch(pz)
                        for c in range(CPT):
                            MM(pz[:], wz[c][:, ds_], hidden[c][:, t0:t0 + FT],
                                             start=(c == 0), stop=(c == CPT - 1))
                        nc.scalar.activation(xi[d][:, t0:t0 + FT], pxi[:], AF.Silu,
                                             bias=cb[:, d:d + 1])
                        nc.scalar.activation(zs[d][:, t0:t0 + FT], pz[:], AF.Silu)

                # x_proj -> dt rows / B / C
                observe(xi[DPT - 1][0:32, T - 32:T], zs[DPT - 1][0:32, T - 32:T],
                        wxp[0][0:32, 0:32])
                dtr = tmp.tile([DT_RANK, T], F32, tag="dtr", name="dtr")
                bm = tmp.tile([D_STATE, T], BF16, tag="bm", name="bm")
                cm = tmp.tile([D_STATE, T], BF16, tag="cm", name="cm")
                for f in range(NF):
                    t0 = f * FT
                    pxd = ps.tile([96, FT], F32, tag="mm", name="mm")
                    if f > 0:
                        observe(cm[0:16, f * FT - 32:f * FT])
                    touch(pxd)
                    for d in range(DPT):
                        MM(pxd[:], wxp[d][:], xi[d][:, t0:t0 + FT],
                                         start=(d == 0), stop=(d == DPT - 1))
                    nc.scalar.activation(dtr[:, t0:t0 + FT], pxd[0:16, :], AF.Identity)
                    nc.scalar.activation(bm[:, t0:t0 + FT], pxd[32:48, :], AF.Identity)
                    nc.scalar.activation(cm[:, t0:t0 + FT], pxd[64:80, :], AF.Identity)

                # selective scan in NQ time-quarters
                yg = [act.tile([P, T], BF16, tag=(f"hid{d}" if d < CPT else f"yg{d}"), name=f"yg{d}") for d in range(DPT)]
                observe(dtr[0:16, 0:32], wdt[0:16, 0:32])
                for q in range(NQ):
                    t0 = q * TQ
                    ts_ = slice(t0, t0 + TQ)
                    delta = [sc1.tile([P, TQ], F32, tag=f"delta{d}", name=f"delta{d}") for d in range(DPT)]
                    wts = [sc1.tile([P, TQ], BF16, tag=f"wts{d}", name=f"wts{d}") for d in range(DPT)]
                    ya = [sc1.tile([P, TQ], BF16, tag=f"ya{d}", name=f"ya{d}") for d in range(DPT)]
                    for d in range(DPT):
                        pdt = ps.tile([P, TQ], F32, tag="mm", name="mm")
                        if d > 0:
                            observe(delta[d - 1][0:32, TQ - 32:TQ])
                        elif q > 0:
                            observe(yg[DPT - 1][0:32, t0 - 32:t0])
                        touch(pdt)
                        MM(pdt[:], wdt[:, d * P:(d + 1) * P], dtr[:, ts_],
                                         start=True, stop=True)
                        esp = sc1.tile([P, TQ], F32, tag=f"esp{d}", name=f"esp{d}")
                        nc.scalar.activation(esp[:], pdt[:], AF.Exp,
                                             bias=dtb[:, d:d + 1])
                        nc.scalar.activation(delta[d][:], esp[:], AF.Ln, bias=1.0)
                        nc.vector.tensor_tensor(wts[d][:], delta[d][:], xi[d][:, ts_],
                                                OP.mult)
                    for n in range(D_STATE):
                        brep = sc2.tile([P, TQ], BF16, tag="brep", name="brep")
                        crep = sc2.tile([P, TQ], BF16, tag="crep", name="crep")
                        bmf = sc2.tile([1, TQ], BF16, tag="bmf", name="bmf")
                        cmf = sc2.tile([1, TQ], BF16, tag="cmf", name="cmf")
                        dma(bmf[0:1, :], bm[n:n + 1, ts_])
                        dma(cmf[0:1, :], cm[n:n + 1, ts_])
                        nc.gpsimd.partition_broadcast(brep[:], bmf[0:1, :])
                        nc.gpsimd.partition_broadcast(crep[:], cmf[0:1, :])
                        for d in range(DPT):
                            ix = n * DPT + d
                            an = sc2.tile([P, TQ], F32, tag="an", name="an")
                            nc.scalar.activation(an[:], delta[d][:], AF.Exp,
                                                 scale=float(-(n + 1)))
                            last_an = an
                            bn = sc2.tile([P, TQ], BF16, tag="bn", name="bn")
                            nc.vector.tensor_tensor(bn[:], wts[d][:], brep[:], OP.mult)
                            hn = sc2.tile([P, TQ], BF16, tag="hn", name="hn")
                            init = 0.0 if q == 0 else hlast[:, ix:ix + 1]
                            nc.vector.tensor_tensor_scan(hn[:], an[:], bn[:], init,
                                                         OP.mult, OP.add)
                            if q < NQ - 1:
                                nc.vector.tensor_copy(hlast[:, ix:ix + 1],
                                                      hn[:, TQ - 1:TQ])
                            chn = sc2.tile([P, TQ], BF16, tag="chn", name="chn")
                            nc.vector.tensor_tensor(chn[:], hn[:], crep[:], OP.mult)
                            if n == 0:
                                nc.vector.tensor_copy(ya[d][:], chn[:])
                            else:
                                nc.vector.tensor_tensor(ya[d][:], ya[d][:], chn[:],
                                                        OP.add)
                    for d in range(DPT):
                        nc.vector.scalar_tensor_tensor(ya[d][:], xi[d][:, ts_],
                                                       dvec[:, d:d + 1], ya[d][:],
                                                       OP.mult, OP.add)
                        nc.vector.tensor_tensor(yg[d][:, ts_], ya[d][:], zs[d][:, ts_],
                                                OP.mult)

                # out_proj + residual -> res2
                observe(yg[DPT - 1][0:32, T - 32:T], wout[0][0:32, 0:32],
                        last_an[0:32, TQ - 32:TQ], delta[DPT - 1][0:32, TQ - 32:TQ])
                res2 = [act.tile([P, T], BF16, tag=f"res2_{c}", name=f"res2_{c}") for c in range(CPT)]
                prev_g = None
                for c in range(CPT):
                    cs_ = slice(c * P, (c + 1) * P)
                    for f in range(NF):
                        t0 = f * FT
                        po = ps.tile([P, FT], F32, tag="mm", name="mm")
                        if prev_g is not None:
                            pc_, pf_ = prev_g
                            observe(res2[pc_][0:32, pf_ * FT + FT - 32:pf_ * FT + FT])
                        touch(po)
                        prev_g = (c, f)
                        for d in range(DPT):
                            MM(po[:], wout[d][:, cs_], yg[d][:, t0:t0 + FT],
                                             start=(d == 0), stop=(d == DPT - 1))
                        nc.vector.tensor_tensor(res2[c][:, t0:t0 + FT], po[:],
                                                xres[c][:, t0:t0 + FT], OP.add)

                # norm2 + mlp
                hid2 = [tmp.tile([P, T], BF16, tag=f"hid2_{c}", name=f"hid2_{c}") for c in range(CPT)]
                rmsnorm_to(hid2, res2)
                hmlp = [tmp.tile([P, T], BF16, tag=("big2" if c == 0 else "big0"), name=f"hmlp{c}") for c in range(CPT)]
                observe(wmlp[0][0:32, 0:32])
                mlp_prev = None
                for c in range(CPT):
                    cs_ = slice(c * P, (c + 1) * P)
                    for f in range(NF):
                        t0 = f * FT
                        pm = ps.tile([P, FT], F32, tag="mm", name="mm")
                        if mlp_prev is not None:
                            pc_, pf_ = mlp_prev
                            observe(hmlp[pc_][0:32, pf_ * FT + FT - 32:pf_ * FT + FT])
                        touch(pm)
                        mlp_prev = (c, f)
                        for c2 in range(CPT):
                            MM(pm[:], wmlp[c2][:, cs_],
                                             hid2[c2][:, t0:t0 + FT],
                                             start=(c2 == 0), stop=(c2 == CPT - 1))
                        nc.scalar.activation(hmlp[c][:, t0:t0 + FT], pm[:], AF.Identity,
                                             bias=mlpb[:, c:c + 1])

                # dense conv (pad=1) + residual -> res
                observe(hmlp[CPT - 1][0:32, 0:32], wcv[0][0][0:32, 0:32])
                res = [act.tile([P, T], F32, tag=f"res_{c}", name=f"res_{c}") for c in range(CPT)]
                cv_prev = None
                for c in range(CPT):
                    cs_ = slice(c * P, (c + 1) * P)
                    for f in range(NF):
                        t0 = f * FT
                        pc = ps.tile([P, FT], F32, tag="mm", name="mm")
                        if cv_prev is not None:
                            pc_, pf_ = cv_prev
                            observe(res[pc_][0:32, pf_ * FT + FT - 32:pf_ * FT + FT])
                        touch(pc)
                        cv_prev = (c, f)
                        first = True
                        for k in range(3):
                            sig = 1 - k
                            lo, hi = t0 - sig, t0 + FT - sig
                            for c2 in range(CPT):
                                lh = wcv[k][c2][:, cs_]
                                if lo >= 0 and hi <= T:
                                    MM(pc[:], lh, hmlp[c2][:, lo:hi],
                                                     start=first, stop=False)
                                elif lo < 0:
                                    MM(pc[:, -lo:], lh, hmlp[c2][:, 0:hi],
                                                     start=first, stop=False)
                                else:
                                    MM(pc[:, :T - lo], lh, hmlp[c2][:, lo:T],
                                                     start=first, stop=False)
                                first = False
                        nc.vector.scalar_tensor_tensor(res[c][:, t0:t0 + FT], pc[:],
                                                       cvb[:, c:c + 1],
                                                       res2[c][:, t0:t0 + FT],
                                                       OP.add, OP.add)

                if l == 0 and do_cc and n_layers > 1:
                    # pair exchange via canonical/flipped slots + parity blend
                    for c in range(CPT):
                        rev = tmp.tile([P, T], F32, tag=f"big{0}", name=f"big{0}")
                        own_c = tmp.tile([P, T], F32, tag=f"big{1}", name=f"big{1}")
                        own_f = tmp.tile([P, T], F32, tag="big2", name="big2")
                        nc.vector.tensor_copy(rev[:], res[c][:, ::-1])
                        nc.vector.tensor_scalar_mul(own_c[:], res[c][:], sa[:, 0:1])
                        nc.vector.scalar_tensor_tensor(own_c[:], rev[:], sb[:, 0:1],
                                                       own_c[:], OP.mult, OP.add)
                        nc.vector.tensor_scalar_mul(own_f[:], res[c][:], sb[:, 0:1])
                        nc.vector.scalar_tensor_tensor(own_f[:], rev[:], sa[:, 0:1],
                                                       own_f[:], OP.mult, OP.add)
                        dma(cc_in[:, (c * 2) * T:(c * 2 + 1) * T], own_c[:])
                        dma(cc_in[:, (c * 2 + 1) * T:(c * 2 + 2) * T], own_f[:])
                    if fake_cc:
                        dma(cc_out[:], cc_in[:])
                    else:
                        nc.gpsimd.collective_compute(
                            "AllReduce", OP.add, replica_groups=groups,
                            ins=[cc_in.opt()], outs=[cc_out.opt()])
                    for c in range(CPT):
                        s_c = tmp.tile([P, T], F32, tag=f"big{0}", name=f"big{0}")
                        s_f = tmp.tile([P, T], F32, tag=f"big{1}", name=f"big{1}")
                        dma(s_c[:], cc_out[:, (c * 2) * T:(c * 2 + 1) * T])
                        dma(s_f[:], cc_out[:, (c * 2 + 1) * T:(c * 2 + 2) * T])
                        nc.vector.tensor_scalar_mul(s_c[:], s_c[:], sa[:, 0:1])
                        nc.vector.scalar_tensor_tensor(s_c[:], s_f[:], sb[:, 0:1],
                                                       s_c[:], OP.mult, OP.add)
                        nc.vector.tensor_tensor(xres[c][:], s_c[:], xres[c][:], OP.add)
                        dma(l1_out[c * P:(c + 1) * P, :], xres[c][:])
                elif l == n_layers - 1:
                    for c in range(CPT):
                        dma(res2_out[c * P:(c + 1) * P, :], res[c][:])
                        dma(l1_out[c * P:(c + 1) * P, :], xres[c][:])

    nc.compile()
    return nc


# ------------------------------------------------------------------- host ---
def _prep_core_inputs(x_stream, lps, par):
    ins = {
        "x_stream": np.ascontiguousarray(x_stream.T).astype(np.float32),
        "ones_lhst": np.ones((P, P), np.float32),
        "par_a": np.full((P, 1), 1.0 - par, np.float32),
        "par_b": np.full((P, 1), float(par), np.float32),
        "epsb": np.full((P, 1), EPS, np.float32),
    }
    for l, (bp, cvw, cvbv) in enumerate(lps):
        m = bp['mixer']
        nw = np.asarray(bp['norm_w'], np.float32)
        ip = np.asarray(m['in_proj'], np.float32)
        cw = np.asarray(m['conv_w'], np.float32)
        wxi = np.empty((3, D_MODEL, D_INNER), np.float32)
        for k in range(3):
            wxi[k] = (ip[:D_INNER] * cw[:, 0, k:k + 1]).T * nw[:, None]
        ins[f"wxi{l}"] = wxi
        ins[f"wz{l}"] = ip[D_INNER:].T * nw[:, None]
        ins[f"cb{l}"] = np.asarray(m['conv_b'], np.float32).reshape(D_INNER, 1)
        xpT = np.asarray(m['x_proj'], np.float32).T  # (512, 48)
        wxp_p = np.zeros((D_INNER, 96), np.float32)
        wxp_p[:, 0:16] = xpT[:, 0:16]
        wxp_p[:, 32:48] = xpT[:, 16:32]
        wxp_p[:, 64:80] = xpT[:, 32:48]
        ins[f"wxp{l}"] = wxp_p
        ins[f"wdt{l}"] = np.asarray(m['dt_proj_w'], np.float32).T.copy()
        ins[f"dtb{l}"] = np.asarray(m['dt_proj_b'], np.float32).reshape(D_INNER, 1)
        A = -np.exp(np.asarray(m['A_log'], np.float32))
        assert np.allclose(A, np.tile(A[:1], (D_INNER, 1)), atol=1e-4)
        assert np.allclose(A[0], -np.arange(1, D_STATE + 1), atol=1e-3)
        ins[f"dvec{l}"] = np.asarray(m['D'], np.float32).reshape(D_INNER, 1)
        ins[f"wout{l}"] = np.asarray(m['out_proj'], np.float32).T.copy()
        n2 = np.asarray(bp['norm2_w'], np.float32)
        ins[f"wmlp{l}"] = np.asarray(bp['mlp_w'], np.float32).T * n2[:, None]
        ins[f"mlpb{l}"] = np.asarray(bp['mlp_b'], np.float32).reshape(D_MODEL, 1)
        cvw = np.asarray(cvw, np.float32)
        wcv = np.empty((3, D_MODEL, D_MODEL), np.float32)
        for k in range(3):
            wcv[k] = cvw[:, :, k].T
        ins[f"wcv{l}"] = wcv
        ins[f"cvb{l}"] = np.asarray(cvbv, np.float32).reshape(D_MODEL, 1)
    return ins


BF16_NAMES = {f"{p}{l}" for l in range(N_LAYERS)
              for p in ("wxi", "wz", "wxp", "wout", "wmlp", "wcv")}


def _make_in_maps(x, params):
    import ml_dtypes
    x = np.asarray(x, np.float32)
    B = x.shape[0]
    in_maps = []
    for b in range(B):
        for par in (0, 1):
            xs = x[b] if par == 0 else x[b, ::-1]
            lps = []
            for l in range(N_LAYERS):
                lp = params[l]
                key = 'fwd' if par == 0 else 'bwd'
                lps.append((lp[f'{key}_blocks'][0], lp[f'{key}_conv_w'],
                            lp[f'{key}_conv_b']))
            ins = _prep_core_inputs(xs, lps, par)
            for k in list(ins):
                if k in BF16_NAMES:
                    ins[k] = ins[k].astype(ml_dtypes.bfloat16)
            in_maps.append(ins)
    return in_maps


def kernel(x, params, _debug=False):
    x = np.asarray(x, np.float32)
    B = x.shape[0]
    nc = build_program()
    in_maps = _make_in_maps(x, params)
    res = run_bass_kernel_spmd(nc, in_maps, core_ids=list(range(8)))
    outs = res.results

    y = np.empty((B, T, D_MODEL), np.float32)
    for b in range(B):
        e, o = outs[2 * b], outs[2 * b + 1]
        comb = e["res2"] + o["res2"][:, ::-1] + e["l1out"]
        y[b] = comb.T
    if _debug:
        return y, outs, res
    return y


# revision 22
# speedup vs baseline: 1.0739x; 1.0739x over previous
"""Bidirectional Mamba (2 layers, B=4, L=2048, D=256) on 8 TRN2 NeuronCores.

Sharding: 8 cores = 4 batches x 2 directions; each core runs one
(batch, direction) stream end-to-end in channel-major (C, T) layout.
 - PE: all matmuls (bf16 lhsT); depthwise causal conv folded into 3
   shifted in_proj taps; dense layer conv as 3 shifted matmuls.
 - Scan: 16 per-n planes, ACT exp(-n*delta) + DVE tensor_tensor_scan
   (state = a*state + b along time), y accumulated in fp32.
 - One pair AllReduce at the layer-1/2 boundary; fwd/bwd time alignment
   via data-driven parity masks (no control flow).
Final combine (res2_f + flip(res2_b) + l1out) on host.
"""

import numpy as np

import concourse.bass as bass
import concourse.bacc as bacc
import concourse.mybir as mybir
from concourse import tile
from concourse.tile_rust import add_dep_helper
from concourse.bass_utils import run_bass_kernel_spmd

F32 = mybir.dt.float32
BF16 = mybir.dt.bfloat16
AF = mybir.ActivationFunctionType
OP = mybir.AluOpType

D_MODEL, D_STATE, D_INNER, DT_RANK = 256, 16, 512, 16
T = 2048
N_LAYERS = 2
P = 128
CPT = D_MODEL // P   # 2
DPT = D_INNER // P   # 4
FT = 512
NF = T // FT         # 4
TQ = 512             # scan quarter
NQ = T // TQ         # 4
EPS = 1e-5


def build_program(n_layers=N_LAYERS, do_cc=True, fake_cc=False):
    nc = bacc.Bacc("TRN2", target_bir_lowering=False, debug=False, num_devices=8)

    def din(name, shape, dt=F32):
        return nc.dram_tensor(name, shape, dt, kind="ExternalInput").ap()

    x_in = din("x_stream", (D_MODEL, T))
    ones_in = din("ones_lhst", (P, P))
    sa_in = din("par_a", (P, 1))
    sb_in = din("par_b", (P, 1))
    eps_in = din("epsb", (P, 1))
    W = {}
    for l in range(n_layers):
        W[l] = dict(
            wxi=din(f"wxi{l}", (3, D_MODEL, D_INNER), BF16),
            wz=din(f"wz{l}", (D_MODEL, D_INNER), BF16),
            cb=din(f"cb{l}", (D_INNER, 1)),
            wxp=din(f"wxp{l}", (D_INNER, 96), BF16),
            wdt=din(f"wdt{l}", (DT_RANK, D_INNER)),
            dtb=din(f"dtb{l}", (D_INNER, 1)),
            dvec=din(f"dvec{l}", (D_INNER, 1)),
            wout=din(f"wout{l}", (D_INNER, D_MODEL), BF16),
            wmlp=din(f"wmlp{l}", (D_MODEL, D_MODEL), BF16),
            mlpb=din(f"mlpb{l}", (D_MODEL, 1)),
            wcv=din(f"wcv{l}", (3, D_MODEL, D_MODEL), BF16),
            cvb=din(f"cvb{l}", (D_MODEL, 1)),
        )
    res2_out = nc.dram_tensor("res2", (D_MODEL, T), F32, kind="ExternalOutput").ap()
    l1_out = nc.dram_tensor("l1out", (D_MODEL, T), F32, kind="ExternalOutput").ap()
    groups = [[0, 1], [2, 3], [4, 5], [6, 7]]

    with tile.TileContext(nc) as tc:
        with (
            tc.tile_pool(name="con", bufs=1) as con,
            tc.tile_pool(name="wp", bufs=1) as wp,
            tc.tile_pool(name="act", bufs=1) as act,
            tc.tile_pool(name="tmp", bufs=1) as tmp,
            tc.tile_pool(name="sc1", bufs=1) as sc1,
            tc.tile_pool(name="sc2", bufs=2) as sc2,
            tc.tile_pool(name="ps", bufs=6, space="PSUM") as ps,
            tc.tile_pool(name="dram", bufs=1, space="DRAM") as dram,
        ):
            def dma(dst, src):
                nc.sync.dma_start(dst, src)

            def load(pool, shape, dt, src, tag):
                t_ = pool.tile(shape, dt, tag=tag)
                dma(t_[:], src)
                return t_

            ones = load(con, [P, P], F32, ones_in, "ones")
            sa = load(con, [P, 1], F32, sa_in, "sa")
            sb = load(con, [P, 1], F32, sb_in, "sb")
            epsb = load(con, [P, 1], F32, eps_in, "epsb")
            hlast = con.tile([P, D_STATE * DPT], F32, tag="hlast", name="hlast")
            obs = ps.tile([32, 32], F32, tag="obs", name="obs", bufs=1)
            pending_obs = []

            def observe(*slices):
                # tiny PE matmuls that pre-observe producer semaphores so real
                # matmuls never carry >1 sync wait (walrus S3_LW limit).
                # Ordered before the next real matmul via nosync deps.
                prev = pending_obs[-1] if pending_obs else None
                for sl in slices:
                    bi = nc.tensor.matmul(obs[0:sl.shape[1], :], sl, sl[:, 0:32],
                                          start=True, stop=True)
                    if prev is not None:
                        add_dep_helper(bi.ins, prev.ins, sync=False,
                                       reason="observer chain order")
                    pending_obs.append(bi)
                    prev = bi

            def touch(ptile):
                bi = nc.tensor.matmul(ptile[0:32, 0:32], ones[0:32, 0:32],
                                      ones[0:32, 0:32], start=True, stop=True,
                                      skip_group_check=True)
                if pending_obs:
                    add_dep_helper(bi.ins, pending_obs[-1].ins, sync=False,
                                   reason="touch after observers")
                pending_obs.append(bi)

            def MM(*args, **kw):
                bi = nc.tensor.matmul(*args, **kw)
                if pending_obs:
                    add_dep_helper(bi.ins, pending_obs[-1].ins, sync=False,
                                   reason="matmul after observers")
                    pending_obs.clear()
                return bi

            cc_in = dram.tile([P, CPT * 2 * T], F32, tag="cc_in", name="cc_in")
            cc_out = dram.tile([P, CPT * 2 * T], F32, tag="cc_out", name="cc_out")

            xres = [act.tile([P, T], F32, tag=f"xres{c}", name=f"xres{c}") for c in range(CPT)]
            for c in range(CPT):
                dma(xres[c][:], x_in[c * P:(c + 1) * P, :])

            def rmsnorm_to(dst, src):
                sq = [tmp.tile([P, T], F32, tag=f"big{c}", name=f"big{c}") for c in range(CPT)]
                for c in range(CPT):
                    nc.scalar.activation(sq[c][:], src[c][:], AF.Square)
                rn = tmp.tile([P, T], F32, tag="big2", name="big2")
                observe(sq[CPT - 1][0:32, 0:32])
                for f in range(NF):
                    s_ = slice(f * FT, (f + 1) * FT)
                    pt_ = ps.tile([P, FT], F32, tag="mm", name="mm")
                    if f > 0:
                        observe(rn[0:32, f * FT - 32:f * FT])
                    touch(pt_)
                    for c in range(CPT):
                        MM(pt_[:], ones[:], sq[c][:, s_],
                                         start=(c == 0), stop=(c == CPT - 1))
                    nc.scalar.activation(rn[:, s_], pt_[:], AF.Ln,
                                         scale=1.0 / D_MODEL, bias=epsb[:, 0:1])
                nc.scalar.activation(rn[:], rn[:], AF.Exp, scale=-0.5)
                for c in range(CPT):
                    nc.vector.tensor_tensor(dst[c][:], src[c][:], rn[:], OP.mult)
                observe(dst[CPT - 1][0:32, 0:32], rn[0:32, 0:32])

            for l in range(n_layers):
                w = W[l]
                wxi = [[load(wp, [P, D_INNER], BF16, w['wxi'][k, c * P:(c + 1) * P, :],
                             f"wxi{k}{c}") for c in range(CPT)] for k in range(3)]
                wz = [load(wp, [P, D_INNER], BF16, w['wz'][c * P:(c + 1) * P, :],
                           f"wz{c}") for c in range(CPT)]
                cb = load(wp, [P, DPT], F32,
                          w['cb'].rearrange("(d p) o -> p (d o)", p=P), "cb")
                wxp = [load(wp, [P, 96], BF16, w['wxp'][d * P:(d + 1) * P, :],
                            f"wxp{d}") for d in range(DPT)]
                wdt = load(wp, [DT_RANK, D_INNER], F32, w['wdt'], "wdt")
                dtb = load(wp, [P, DPT], F32,
                           w['dtb'].rearrange("(d p) o -> p (d o)", p=P), "dtb")
                dvec = load(wp, [P, DPT], F32,
                            w['dvec'].rearrange("(d p) o -> p (d o)", p=P), "dvec")
                wout = [load(wp, [P, D_MODEL], BF16, w['wout'][d * P:(d + 1) * P, :],
                             f"wout{d}") for d in range(DPT)]
                wmlp = [load(wp, [P, D_MODEL], BF16, w['wmlp'][c * P:(c + 1) * P, :],
                             f"wmlp{c}") for c in range(CPT)]
                mlpb = load(wp, [P, CPT], F32,
                            w['mlpb'].rearrange("(d p) o -> p (d o)", p=P), "mlpb")
                wcv = [[load(wp, [P, D_MODEL], BF16, w['wcv'][k, c * P:(c + 1) * P, :],
                             f"wcv{k}{c}") for c in range(CPT)] for k in range(3)]
                cvb = load(wp, [P, CPT], F32,
                           w['cvb'].rearrange("(d p) o -> p (d o)", p=P), "cvb")

                hidden = [act.tile([P, T], BF16, tag=f"hid{c}", name=f"hid{c}") for c in range(CPT)]
                rmsnorm_to(hidden, xres)

                # in_proj (+ folded depthwise causal conv) -> silu -> xi ; z -> silu -> zs
                xi = [act.tile([P, T], BF16, tag=f"xi{d}", name=f"xi{d}") for d in range(DPT)]
                zs = [act.tile([P, T], BF16, tag=f"zs{d}", name=f"zs{d}") for d in range(DPT)]
                observe(wxi[0][0][0:32, 0:32])
                prev_g = None
                for d in range(DPT):
                    ds_ = slice(d * P, (d + 1) * P)
                    for f in range(NF):
                        t0 = f * FT
                        pxi = ps.tile([P, FT], F32, tag="mm", name="mm")
                        if prev_g is not None:
                            pd_, pf_ = prev_g
                            observe(zs[pd_][0:32, pf_ * FT + FT - 32:pf_ * FT + FT])
                        touch(pxi)
                        prev_g = (d, f)
                        first = True
                        for k in range(3):
                            sig = 2 - k
                            for c in range(CPT):
                                lh = wxi[k][c][:, ds_]
                                if t0 - sig >= 0:
                                    MM(pxi[:], lh,
                                                     hidden[c][:, t0 - sig:t0 + FT - sig],
                                                     start=first, stop=False)
                                else:
                                    MM(pxi[:, sig:], lh,
                                                     hidden[c][:, 0:FT - sig],
                                                     start=first, stop=False)
                                first = False
                        pz = ps.tile([P, FT], F32, tag="mm", name="mm")
                        touch(pz)
                        for c in range(CPT):
                            MM(pz[:], wz[c][:, ds_], hidden[c][:, t0:t0 + FT],
                                             start=(c == 0), stop=(c == CPT - 1))
                        nc.scalar.activation(xi[d][:, t0:t0 + FT], pxi[:], AF.Silu,
                                             bias=cb[:, d:d + 1])
                        nc.scalar.activation(zs[d][:, t0:t0 + FT], pz[:], AF.Silu)

                # x_proj -> dt rows / B / C
                observe(xi[DPT - 1][0:32, T - 32:T], zs[DPT - 1][0:32, T - 32:T],
                        wxp[0][0:32, 0:32])
                dtr = tmp.tile([DT_RANK, T], F32, tag="dtr", name="dtr")
                bm = tmp.tile([D_STATE, T], BF16, tag="bm", name="bm")
                cm = tmp.tile([D_STATE, T], BF16, tag="cm", name="cm")
                for f in range(NF):
                    t0 = f * FT
                    pxd = ps.tile([96, FT], F32, tag="mm", name="mm")
                    if f > 0:
                        observe(cm[0:16, f * FT - 32:f * FT])
                    touch(pxd)
                    for d in range(DPT):
                        MM(pxd[:], wxp[d][:], xi[d][:, t0:t0 + FT],
                                         start=(d == 0), stop=(d == DPT - 1))
                    nc.scalar.activation(dtr[:, t0:t0 + FT], pxd[0:16, :], AF.Identity)
                    nc.scalar.activation(bm[:, t0:t0 + FT], pxd[32:48, :], AF.Identity)
                    nc.scalar.activation(cm[:, t0:t0 + FT], pxd[64:80, :], AF.Identity)

                # selective scan in NQ time-quarters
                yg = [act.tile([P, T], BF16, tag=(f"hid{d}" if d < CPT else f"yg{d}"), name=f"yg{d}") for d in range(DPT)]
                observe(dtr[0:16, 0:32], wdt[0:16, 0:32])
                for q in range(NQ):
                    t0 = q * TQ
                    ts_ = slice(t0, t0 + TQ)
                    delta = [sc1.tile([P, TQ], F32, tag=f"delta{d}", name=f"delta{d}") for d in range(DPT)]
                    wts = [sc1.tile([P, TQ], BF16, tag=f"wts{d}", name=f"wts{d}") for d in range(DPT)]
                    ya = [sc1.tile([P, TQ], BF16, tag=f"ya{d}", name=f"ya{d}") for d in range(DPT)]
                    yo = [sc1.tile([P, TQ], BF16, tag=f"yo{d}", name=f"yo{d}") for d in range(DPT)]
                    for d in range(DPT):
                        pdt = ps.tile([P, TQ], F32, tag="mm", name="mm")
                        if d > 0:
                            observe(delta[d - 1][0:32, TQ - 32:TQ])
                        elif q > 0:
                            observe(yg[DPT - 1][0:32, t0 - 32:t0])
                        touch(pdt)
                        MM(pdt[:], wdt[:, d * P:(d + 1) * P], dtr[:, ts_],
                                         start=True, stop=True)
                        esp = sc1.tile([P, TQ], F32, tag=f"esp{d}", name=f"esp{d}")
                        nc.scalar.activation(esp[:], pdt[:], AF.Exp,
                                             bias=dtb[:, d:d + 1])
                        nc.scalar.activation(delta[d][:], esp[:], AF.Ln, bias=1.0)
                        nc.vector.tensor_tensor(wts[d][:], delta[d][:], xi[d][:, ts_],
                                                OP.mult)
                    for n in range(D_STATE):
                        brep = sc2.tile([P, TQ], BF16, tag="brep", name="brep")
                        crep = sc2.tile([P, TQ], BF16, tag="crep", name="crep")
                        bmf = sc2.tile([1, TQ], BF16, tag="bmf", name="bmf")
                        cmf = sc2.tile([1, TQ], BF16, tag="cmf", name="cmf")
                        dma(bmf[0:1, :], bm[n:n + 1, ts_])
                        dma(cmf[0:1, :], cm[n:n + 1, ts_])
                        nc.gpsimd.partition_broadcast(brep[:], bmf[0:1, :])
                        nc.gpsimd.partition_broadcast(crep[:], cmf[0:1, :])
                        for d in range(DPT):
                            ix = n * DPT + d
                            an = sc2.tile([P, TQ], F32, tag="an", name="an")
                            nc.scalar.activation(an[:], delta[d][:], AF.Exp,
                                                 scale=float(-(n + 1)))
                            last_an = an
                            bn = sc2.tile([P, TQ], BF16, tag="bn", name="bn")
                            beng = nc.vector if n % 2 == 0 else nc.gpsimd
                            beng.tensor_tensor(bn[:], wts[d][:], brep[:], OP.mult)
                            hn = sc2.tile([P, TQ], BF16, tag="hn", name="hn")
                            init = 0.0 if q == 0 else hlast[:, ix:ix + 1]
                            nc.vector.tensor_tensor_scan(hn[:], an[:], bn[:], init,
                                                         OP.mult, OP.add)
                            if q < NQ - 1:
                                nc.vector.tensor_copy(hlast[:, ix:ix + 1],
                                                      hn[:, TQ - 1:TQ])
                            chn = sc2.tile([P, TQ], BF16, tag="chn", name="chn")
                            nc.vector.tensor_tensor(chn[:], hn[:], crep[:], OP.mult)
                            if n == 0:
                                nc.vector.tensor_copy(ya[d][:], chn[:])
                            elif n == 1:
                                nc.gpsimd.tensor_copy(yo[d][:], chn[:])
                            elif n % 2 == 0:
                                nc.vector.tensor_tensor(ya[d][:], ya[d][:], chn[:],
                                                        OP.add)
                            else:
                                nc.gpsimd.tensor_tensor(yo[d][:], yo[d][:], chn[:],
                                                        OP.add)
                    for d in range(DPT):
                        nc.vector.tensor_tensor(ya[d][:], ya[d][:], yo[d][:], OP.add)
                        nc.vector.scalar_tensor_tensor(ya[d][:], xi[d][:, ts_],
                                                       dvec[:, d:d + 1], ya[d][:],
                                                       OP.mult, OP.add)
                        nc.vector.tensor_tensor(yg[d][:, ts_], ya[d][:], zs[d][:, ts_],
                                                OP.mult)

                # out_proj + residual -> res2
                observe(yg[DPT - 1][0:32, T - 32:T], wout[0][0:32, 0:32],
                        last_an[0:32, TQ - 32:TQ], delta[DPT - 1][0:32, TQ - 32:TQ])
                res2 = [act.tile([P, T], BF16, tag=f"res2_{c}", name=f"res2_{c}") for c in range(CPT)]
                prev_g = None
                for c in range(CPT):
                    cs_ = slice(c * P, (c + 1) * P)
                    for f in range(NF):
                        t0 = f * FT
                        po = ps.tile([P, FT], F32, tag="mm", name="mm")
                        if prev_g is not None:
                            pc_, pf_ = prev_g
                            observe(res2[pc_][0:32, pf_ * FT + FT - 32:pf_ * FT + FT])
                        touch(po)
                        prev_g = (c, f)
                        for d in range(DPT):
                            MM(po[:], wout[d][:, cs_], yg[d][:, t0:t0 + FT],
                                             start=(d == 0), stop=(d == DPT - 1))
                        nc.vector.tensor_tensor(res2[c][:, t0:t0 + FT], po[:],
                                                xres[c][:, t0:t0 + FT], OP.add)

                # norm2 + mlp
                hid2 = [tmp.tile([P, T], BF16, tag=f"hid2_{c}", name=f"hid2_{c}") for c in range(CPT)]
                rmsnorm_to(hid2, res2)
                hmlp = [tmp.tile([P, T], BF16, tag=("big2" if c == 0 else "big0"), name=f"hmlp{c}") for c in range(CPT)]
                observe(wmlp[0][0:32, 0:32])
                mlp_prev = None
                for c in range(CPT):
                    cs_ = slice(c * P, (c + 1) * P)
                    for f in range(NF):
                        t0 = f * FT
                        pm = ps.tile([P, FT], F32, tag="mm", name="mm")
                        if mlp_prev is not None:
                            pc_, pf_ = mlp_prev
                            observe(hmlp[pc_][0:32, pf_ * FT + FT - 32:pf_ * FT + FT])
                        touch(pm)
                        mlp_prev = (c, f)
                        for c2 in range(CPT):
                            MM(pm[:], wmlp[c2][:, cs_],
                                             hid2[c2][:, t0:t0 + FT],
                                             start=(c2 == 0), stop=(c2 == CPT - 1))
                        nc.scalar.activation(hmlp[c][:, t0:t0 + FT], pm[:], AF.Identity,
                                             bias=mlpb[:, c:c + 1])

                # dense conv (pad=1) + residual -> res
                observe(hmlp[CPT - 1][0:32, 0:32], wcv[0][0][0:32, 0:32])
                res = [act.tile([P, T], F32, tag=f"res_{c}", name=f"res_{c}") for c in range(CPT)]
                cv_prev = None
                for c in range(CPT):
                    cs_ = slice(c * P, (c + 1) * P)
                    for f in range(NF):
                        t0 = f * FT
                        pc = ps.tile([P, FT], F32, tag="mm", name="mm")
                        if cv_prev is not None:
                            pc_, pf_ = cv_prev
                            observe(res[pc_][0:32, pf_ * FT + FT - 32:pf_ * FT + FT])
                        touch(pc)
                        cv_prev = (c, f)
                        first = True
                        for k in range(3):
                            sig = 1 - k
                            lo, hi = t0 - sig, t0 + FT - sig
                            for c2 in range(CPT):
                                lh = wcv[k][c2][:, cs_]
                                if lo >= 0 and hi <= T:
                                    MM(pc[:], lh, hmlp[c2][:, lo:hi],
                                                     start=first, stop=False)
                                elif lo < 0:
                                    MM(pc[:, -lo:], lh, hmlp[c2][:, 0:hi],
                                                     start=first, stop=False)
                                else:
                                    MM(pc[:, :T - lo], lh, hmlp[c2][:, lo:T],
                                                     start=first, stop=False)
                                first = False
                        nc.vector.scalar_tensor_tensor(res[c][:, t0:t0 + FT], pc[:],
                                                       cvb[:, c:c + 1],
                                                       res2[c][:, t0:t0 + FT],
                                                       OP.add, OP.add)

                if l == 0 and do_cc and n_layers > 1:
                    # pair exchange via canonical/flipped slots + parity blend
                    for c in range(CPT):
                        rev = tmp.tile([P, T], F32, tag=f"big{0}", name=f"big{0}")
                        own_c = tmp.tile([P, T], F32, tag=f"big{1}", name=f"big{1}")
                        own_f = tmp.tile([P, T], F32, tag="big2", name="big2")
                        nc.vector.tensor_copy(rev[:], res[c][:, ::-1])
                        nc.vector.tensor_scalar_mul(own_c[:], res[c][:], sa[:, 0:1])
                        nc.vector.scalar_tensor_tensor(own_c[:], rev[:], sb[:, 0:1],
                                                       own_c[:], OP.mult, OP.add)
                        nc.vector.tensor_scalar_mul(own_f[:], res[c][:], sb[:, 0:1])
                        nc.vector.scalar_tensor_tensor(own_f[:], rev[:], sa[:, 0:1],
                                                       own_f[:], OP.mult, OP.add)
                        dma(cc_in[:, (c * 2) * T:(c * 2 + 1) * T], own_c[:])
                        dma(cc_in[:, (c * 2 + 1) * T:(c * 2 + 2) * T], own_f[:])
                    if fake_cc:
                        dma(cc_out[:], cc_in[:])
                    else:
                        nc.gpsimd.collective_compute(
                            "AllReduce", OP.add, replica_groups=groups,
                            ins=[cc_in.opt()], outs=[cc_out.opt()])
                    for c in range(CPT):
                        s_c = tmp.tile([P, T], F32, tag=f"big{0}", name=f"big{0}")
                        s_f = tmp.tile([P, T], F32, tag=f"big{1}", name=f"big{1}")
                        dma(s_c[:], cc_out[:, (c * 2) * T:(c * 2 + 1) * T])
                        dma(s_f[:], cc_out[:, (c * 2 + 1) * T:(c * 2 + 2) * T])
                        nc.vector.tensor_scalar_mul(s_c[:], s_c[:], sa[:, 0:1])
                        nc.vector.scalar_tensor_tensor(s_c[:], s_f[:], sb[:, 0:1],
                                                       s_c[:], OP.mult, OP.add)
                        nc.vector.tensor_tensor(xres[c][:], s_c[:], xres[c][:], OP.add)
                        dma(l1_out[c * P:(c + 1) * P, :], xres[c][:])
                elif l == n_layers - 1:
                    for c in range(CPT):
                        dma(res2_out[c * P:(c + 1) * P, :], res[c][:])
                        dma(l1_out[c * P:(c + 1) * P, :], xres[c][:])

    nc.compile()
    return nc


# ------------------------------------------------------------------- host ---
def _prep_core_inputs(x_stream, lps, par):
    ins = {
        "x_stream": np.ascontiguousarray(x_stream.T).astype(np.float32),
        "ones_lhst": np.ones((P, P), np.float32),
        "par_a": np.full((P, 1), 1.0 - par, np.float32),
        "par_b": np.full((P, 1), float(par), np.float32),
        "epsb": np.full((P, 1), EPS, np.float32),
    }
    for l, (bp, cvw, cvbv) in enumerate(lps):
        m = bp['mixer']
        nw = np.asarray(bp['norm_w'], np.float32)
        ip = np.asarray(m['in_proj'], np.float32)
        cw = np.asarray(m['conv_w'], np.float32)
        wxi = np.empty((3, D_MODEL, D_INNER), np.float32)
        for k in range(3):
            wxi[k] = (ip[:D_INNER] * cw[:, 0, k:k + 1]).T * nw[:, None]
        ins[f"wxi{l}"] = wxi
        ins[f"wz{l}"] = ip[D_INNER:].T * nw[:, None]
        ins[f"cb{l}"] = np.asarray(m['conv_b'], np.float32).reshape(D_INNER, 1)
        xpT = np.asarray(m['x_proj'], np.float32).T  # (512, 48)
        wxp_p = np.zeros((D_INNER, 96), np.float32)
        wxp_p[:, 0:16] = xpT[:, 0:16]
        wxp_p[:, 32:48] = xpT[:, 16:32]
        wxp_p[:, 64:80] = xpT[:, 32:48]
        ins[f"wxp{l}"] = wxp_p
        ins[f"wdt{l}"] = np.asarray(m['dt_proj_w'], np.float32).T.copy()
        ins[f"dtb{l}"] = np.asarray(m['dt_proj_b'], np.float32).reshape(D_INNER, 1)
        A = -np.exp(np.asarray(m['A_log'], np.float32))
        assert np.allclose(A, np.tile(A[:1], (D_INNER, 1)), atol=1e-4)
        assert np.allclose(A[0], -np.arange(1, D_STATE + 1), atol=1e-3)
        ins[f"dvec{l}"] = np.asarray(m['D'], np.float32).reshape(D_INNER, 1)
        ins[f"wout{l}"] = np.asarray(m['out_proj'], np.float32).T.copy()
        n2 = np.asarray(bp['norm2_w'], np.float32)
        ins[f"wmlp{l}"] = np.asarray(bp['mlp_w'], np.float32).T * n2[:, None]
        ins[f"mlpb{l}"] = np.asarray(bp['mlp_b'], np.float32).reshape(D_MODEL, 1)
        cvw = np.asarray(cvw, np.float32)
        wcv = np.empty((3, D_MODEL, D_MODEL), np.float32)
        for k in range(3):
            wcv[k] = cvw[:, :, k].T
        ins[f"wcv{l}"] = wcv
        ins[f"cvb{l}"] = np.asarray(cvbv, np.float32).reshape(D_MODEL, 1)
    return ins


BF16_NAMES = {f"{p}{l}" for l in range(N_LAYERS)
              for p in ("wxi", "wz", "wxp", "wout", "wmlp", "wcv")}


def _make_in_maps(x, params):
    import ml_dtypes
    x = np.asarray(x, np.float32)
    B = x.shape[0]
    in_maps = []
    for b in range(B):
        for par in (0, 1):
            xs = x[b] if par == 0 else x[b, ::-1]
            lps = []
            for l in range(N_LAYERS):
                lp = params[l]
                key = 'fwd' if par == 0 else 'bwd'
                lps.append((lp[f'{key}_blocks'][0], lp[f'{key}_conv_w'],
                            lp[f'{key}_conv_b']))
            ins = _prep_core_inputs(xs, lps, par)
            for k in list(ins):
                if k in BF16_NAMES:
                    ins[k] = ins[k].astype(ml_dtypes.bfloat16)
            in_maps.append(ins)
    return in_maps


def kernel(x, params, _debug=False):
    x = np.asarray(x, np.float32)
    B = x.shape[0]
    nc = build_program()
    in_maps = _make_in_maps(x, params)
    res = run_bass_kernel_spmd(nc, in_maps, core_ids=list(range(8)))
    outs = res.results

    y = np.empty((B, T, D_MODEL), np.float32)
    for b in range(B):
        e, o = outs[2 * b], outs[2 * b + 1]
        comb = e["res2"] + o["res2"][:, ::-1] + e["l1out"]
        y[b] = comb.T
    if _debug:
        return y, outs, res
    return y
